# revision 31
# baseline (speedup 1.0000x reference)
"""GAT (3-layer graph attention + final linear) Trainium2 Bass kernel, v2.

Problem: B=4 graphs, N=2048 atoms, D=128, H=256.
  per layer: h = relu(x @ W.T + b); e_ij = leaky_relu(f1_i + f2_j, 0.01)
  masked by adj; att = softmax_j(e); x = x + att @ h.
  final: relu(x @ Wt.T + bt).

Sharding: 8 cores; core c -> (graph b=c//2, row-half s=c%2). Each core
computes attention for its own 1024 rows (i), over all 2048 atoms (j).

Key design (vs v1 baseline at 607us):
  - Rank-1 softmax restructure: divide row i of the numerator by
    exp(f1_i) (cancels in softmax).  With z = f1_i + f2_j,
      exp(leaky(z)) / e^{f1_i + C} = max(u_j, v_j * w_i)
    where u = exp(f2 - C), v = exp(.01 f2), w = exp(-.99 f1 - C),
    C = local max f2 (any row-constant cancels; C only bounds ranges).
    So the NxN attention numerator needs NO exp, NO logit matmul and NO
    mask-preload matmul: one fused DVE tensor_scalar (mult by v-col,
    max with u-col over a broadcast w tile) plus one DVE mask multiply.
    PE streams each masked-prob tile twice (aggregation + row-sum).
  - All dense matmuls in float32r (1 cy/row vs fp32's 2x4) or bf16.
  - Mask shipped from HOST as ready-to-use transposed bf16 0/1 tiles
    (adj[b][rows,:].T) - no device-side int32 convert/transpose pass,
    and half the HBM traffic of int32.
  - Local-j index remap (host permutes mask columns so each core's own
    atoms come first) makes the program fully SPMD-uniform, and the
    inter-layer exchange is a pair AllReduce(add) of the updated own
    rows; peer half = pairsum - own (one DVE subtract). Own-half
    attention work overlaps the collective on every core.
  - A tiny dummy AllReduce at kernel start absorbs the ~80us
    first-collective warm-up under layer-0 compute.
  - x state kept transposed end-to-end; normalization applied in
    transposed space via a gpsimd partition_broadcast of 1/rowsum.
"""

import numpy as np
import ml_dtypes

import concourse.bass as bass
import concourse.mybir as mybir
import concourse.tile as tile
from concourse import library_config, masks
from concourse.bass_utils import run_bass_kernel_spmd

P = 128
F32 = mybir.dt.float32
F32R = mybir.dt.float32r
BF16 = mybir.dt.bfloat16
AF = mybir.ActivationFunctionType
OP = mybir.AluOpType

# Static exponent shift: bounds exp() ranges (cancels exactly in softmax).
# f2 stays well under this for the GAT data distribution (observed max ~25).
CBIAS = -24.0


def _legalize_waits(nc, dma_limit=1, engine_limit=1):
    """Walrus can encode only 1 sem wait on a DMA instruction and ~2 on an
    engine instruction. Move excess waits onto standalone EventSemaphore
    instructions (1 wait each) inserted just before the offender on the
    same engine."""
    counter = [0]

    def split(ins):
        si = ins.sync_info
        if si is None:
            return None
        limit = dma_limit if type(ins).__name__.startswith("InstDMA") \
            else engine_limit
        waits = list(si.on_wait)
        if len(waits) <= limit:
            return None
        keep = waits[-limit:] if limit > 0 else []
        extra = waits[:-limit] if limit > 0 else waits
        evs = []
        for w in extra:
            counter[0] += 1
            evs.append(mybir.InstEventSemaphore(
                name=f"evsplit{counter[0]}", engine=ins.engine,
                sync_info=mybir.SyncInfo(on_wait=[w], on_update=[])))
        ins.sync_info = mybir.SyncInfo(on_wait=keep,
                                       on_update=list(si.on_update))
        return evs

    for f in nc.m.functions:
        for blk in f.blocks:
            new_list = []
            changed = False
            for ins in blk.instructions:
                evs = split(ins)
                if evs:
                    new_list.extend(evs)
                    changed = True
                new_list.append(ins)
            if changed:
                blk.instructions = new_list


def r32(ap):
    return ap.bitcast(F32R)


def build_gat_nc(N, NS, D, H, num_cores, pair_groups, nlayers=3,
                 legalize=True):
    assert D == P and NS % 512 == 0 and N % 512 == 0
    nj = N // P          # j tiles over all atoms (local order: own first)
    njo = nj // 2        # own j tiles
    nch = NS // 512      # 512-chunks over own rows / one atom half
    nH = H // P

    nc = bass.Bass("TRN2", target_bir_lowering=False, debug=False,
                   num_devices=num_cores)

    # ---- I/O ----
    xTs_in = nc.dram_tensor("xTs", [P, NS], F32, kind="ExternalInput")
    xTo_in = nc.dram_tensor("xTo", [P, NS], F32, kind="ExternalInput")
    maskT_in = nc.dram_tensor("maskT", [N, NS], BF16, kind="ExternalInput")
    WT_in = [nc.dram_tensor(f"WT{l}", [D, D], F32, kind="ExternalInput")
             for l in range(nlayers)]
    bv_in = [nc.dram_tensor(f"bv{l}", [D, 1], F32, kind="ExternalInput")
             for l in range(nlayers)]
    av_in = [nc.dram_tensor(f"av{l}", [D, 33], F32, kind="ExternalInput")
             for l in range(nlayers)]
    WtT_in = nc.dram_tensor("WtT", [D, H], F32, kind="ExternalInput")
    btp_in = nc.dram_tensor("btp", [P, nH], F32, kind="ExternalInput")
    out_ext = nc.dram_tensor("out_s", [H, NS], F32, kind="ExternalOutput")

    # DRAM bounce buffers for the pair AllReduce of updated own rows
    ag_in = [nc.dram_tensor(f"ag_in{l}", [P, NS], BF16)
             for l in range(nlayers - 1)]
    ag_out = [nc.dram_tensor(f"ag_out{l}", [P, NS], BF16)
              for l in range(nlayers - 1)]
    dw_in = nc.dram_tensor("dw_in", [1, P], BF16)
    dw_out = nc.dram_tensor("dw_out", [1, P], BF16)
    fd = [nc.dram_tensor(f"fd{i}", [N // P // 2, P], F32)
          for i in range(2 * nlayers)]

    with tile.TileContext(nc) as tc:
        import contextlib
        ctx = contextlib.ExitStack()
        with ctx:
            persist = ctx.enter_context(tc.tile_pool(name="persist", bufs=1))
            hTp = ctx.enter_context(tc.tile_pool(name="hTp", bufs=2))
            hnp = ctx.enter_context(tc.tile_pool(name="hnp", bufs=2))
            xp = ctx.enter_context(tc.tile_pool(name="xp", bufs=2))
            rowp = ctx.enter_context(tc.tile_pool(name="rowp", bufs=2))
            colp = ctx.enter_context(tc.tile_pool(name="colp", bufs=2))
            bigp = ctx.enter_context(tc.tile_pool(name="bigp", bufs=2))
            pmp = ctx.enter_context(tc.tile_pool(name="pmp", bufs=6))
            outp = ctx.enter_context(tc.tile_pool(name="outp", bufs=2))
            smallp = ctx.enter_context(tc.tile_pool(name="smallp", bufs=4))
            rot = ctx.enter_context(
                tc.tile_pool(name="rot", bufs=4, space="PSUM"))
            attp = ctx.enter_context(
                tc.tile_pool(name="attp", bufs=1, space="PSUM"))
            spp = ctx.enter_context(
                tc.tile_pool(name="spp", bufs=1, space="PSUM"))

            ident = persist.tile([P, P], F32)
            masks.make_identity(nc, ident[:])
            identR = persist.tile([P, P], F32R)
            nc.vector.tensor_copy(identR[:], ident[:])
            onesf = persist.tile([33, P], F32)
            nc.vector.memset(onesf[:], 1.0)
            onesr = persist.tile([33, P], F32R)
            nc.vector.tensor_copy(onesr[:], onesf[:])
            onesm = persist.tile([P, P], BF16)
            nc.vector.memset(onesm[:], 1.0)
            identb = persist.tile([P, P], BF16)
            masks.make_identity(nc, identb[:])
            # PE warm-up filler: no data deps, finishes before the first
            # input DMA lands (~12us); keeps the PE clock ramping instead
            # of cold-idling through NEFF queue init.
            warm = rot.tile([P, 512], BF16, name="warmps", tag="rot")
            for w in range(28):
                nc.tensor.transpose(warm[:, (w % 4) * P:(w % 4 + 1) * P],
                                    identb[:], identb[:])
            cbias = persist.tile([P, 1], F32)
            nc.vector.memset(cbias[:], CBIAS)

            # ---- dummy collective to warm the CC stream ----
            nc.gpsimd.collective_compute(
                "AllReduce", OP.add, replica_groups=pair_groups,
                ins=[dw_in.ap()], outs=[dw_out.ap()])

            # ---- persistent state ----
            maskT = [persist.tile([P, NS], BF16, name=f"maskT{j}",
                                  tag=f"maskT{j}") for j in range(nj)]

            # ---- initial x state (own + peer halves, transposed) ----
            xTs_d = persist.tile([P, NS], F32, name="xTs_d", tag="xTs_d")
            nc.sync.dma_start(xTs_d[:], xTs_in.ap())
            xTo_d = persist.tile([P, NS], F32, name="xTo_d", tag="xTo_d")
            nc.sync.dma_start(xTo_d[:], xTo_in.ap())
            xTs = xp.tile([P, NS], F32R, name="xTs0", tag="xTs")
            nc.vector.tensor_copy(xTs[:], xTs_d[:])
            x1 = xp.tile([P, NS], F32R, name="xTo0", tag="x1")
            nc.vector.tensor_copy(x1[:], xTo_d[:])

            # weights: DMA + DVE launder so matmuls see single-writer tiles
            WT_d = [persist.tile([D, D], F32, name=f"WTd{l}", tag=f"WTd{l}")
                    for l in range(nlayers)]
            bv_d = [persist.tile([D, 1], F32, name=f"bvd{l}", tag=f"bvd{l}")
                    for l in range(nlayers)]
            av_d = [persist.tile([D, 33], F32, name=f"avd{l}", tag=f"avd{l}")
                    for l in range(nlayers)]
            WtT_d = persist.tile([D, H], F32)
            btp_d = persist.tile([P, nH], F32)
            WT = [persist.tile([D, D], F32R, name=f"WTl{l}", tag=f"WTl{l}")
                  for l in range(nlayers)]
            bv = [persist.tile([D, 1], F32, name=f"bvl{l}", tag=f"bvl{l}")
                  for l in range(nlayers)]
            av = [persist.tile([D, 33], F32R, name=f"avl{l}", tag=f"avl{l}")
                  for l in range(nlayers)]
            WtTt = persist.tile([D, H], F32R)
            btpt = persist.tile([P, nH], F32)
            for l in range(nlayers):
                nc.sync.dma_start(WT_d[l][:], WT_in[l].ap())
                nc.sync.dma_start(bv_d[l][:], bv_in[l].ap())
                nc.sync.dma_start(av_d[l][:], av_in[l].ap())
                nc.vector.tensor_copy(WT[l][:], WT_d[l][:])
                nc.vector.tensor_copy(bv[l][:], bv_d[l][:])
                nc.vector.tensor_copy(av[l][:], av_d[l][:])
            nc.sync.dma_start(WtT_d[:], WtT_in.ap())
            nc.sync.dma_start(btp_d[:], btp_in.ap())
            nc.vector.tensor_copy(WtTt[:], WtT_d[:])
            nc.vector.tensor_copy(btpt[:], btp_d[:])
            # masks after x/weights; own half first; spread across queues
            for j in range(nj):
                q = nc.sync if j < njo else nc.scalar
                q.dma_start(maskT[j][:], maskT_in.ap()[j * P:(j + 1) * P, :])

            for l in range(nlayers):
                last = l == nlayers - 1
                hT = hTp.tile([P, N], F32R, name=f"hT{l}", tag="hT")
                hTb = hTp.tile([P, N], BF16, name=f"hTb{l}", tag="hTb")
                frows = rowp.tile([33, N], F32R, name=f"frows{l}", tag="frows")
                ucols = colp.tile([P, nj], F32, name=f"uc{l}", tag="uc")
                vcols = colp.tile([P, nj], F32, name=f"vc{l}", tag="vc")
                hnat = hnp.tile([P, N], BF16, name=f"hnat{l}", tag="hnat")
                psAT = attp.tile([P, NS], F32, name=f"psAT{l}", tag="att")
                psSb = spp.tile([P, NS], F32, name=f"psS{l}", tag="s")
                Wb = bigp.tile([P, NS], BF16, name=f"Wb{l}", tag="Wb")

                # process halves: 0 = own (local data), 1 = peer
                for half in range(2):
                    xsrc = xTs if half == 0 else x1
                    aoff = half * NS  # atom offset in local-j space
                    # chunk-pipelined: h -> f -> (Wb) -> hnat per 512 chunk
                    for ch in range(nch):
                        sl = slice(aoff + ch * 512, aoff + (ch + 1) * 512)
                        ps = rot.tile([P, 512], F32, name=f"h{l}_{half}{ch}",
                                      tag="rot")
                        nc.tensor.matmul(ps[:], WT[l][:],
                                         xsrc[:, ch * 512:(ch + 1) * 512],
                                         start=True, stop=True)
                        nc.scalar.activation(hT[:, sl], ps[:], AF.Relu,
                                             bias=bv[l][:])
                        nc.scalar.activation(hTb[:, sl], ps[:], AF.Relu,
                                             bias=bv[l][:])
                        psf = rot.tile([33, 512], F32,
                                       name=f"f{l}_{half}{ch}", tag="rot")
                        nc.tensor.matmul(psf[:], av[l][:], hT[:, sl],
                                         start=True, stop=True)
                        nc.scalar.copy(frows[:, sl], psf[:])
                        if half == 0:
                            # Wb[p, i] = exp(-0.99 f1_i - C): fp32r K=1
                            # broadcast of raw f1, exp fused in ACT copy.
                            slo = slice(ch * 512, (ch + 1) * 512)
                            psw = rot.tile([P, 512], F32,
                                           name=f"psw{l}_{ch}", tag="rot")
                            nc.tensor.matmul(psw[:], onesr[32:33, :],
                                             frows[32:33, slo],
                                             start=True, stop=True)
                            nc.scalar.activation(Wb[:, slo], psw[:], AF.Exp,
                                                 bias=cbias[:], scale=-0.99)
                        pst = rot.tile([P, 512], BF16,
                                       name=f"ht{l}_{half}{ch}", tag="rot")
                        for q in range(4):
                            j = half * njo + ch * 4 + q
                            nc.tensor.transpose(pst[:, q * P:(q + 1) * P],
                                                hTb[:, j * P:(j + 1) * P],
                                                identb[:])
                        nc.scalar.copy(
                            hnat[:, aoff + ch * 512:aoff + (ch + 1) * 512],
                            pst[:])

                    csl = slice(half * njo, (half + 1) * njo)
                    psc = rot.tile([P, njo], F32, name=f"psc{l}_{half}",
                                   tag="rot")
                    if half == 0 and l > 0:
                        # own half: f2 row -> cols via DRAM-bounce reshape +
                        # one 8-row transpose (latency-tolerant: overlaps the
                        # previous layer's peer inner loop). Layer 0 has no
                        # such overlap window - use direct transposes there.
                        fdt = fd[2 * l + half]
                        nc.sync.dma_start(
                            fdt.ap().rearrange("a b -> (a b)").unsqueeze(0),
                            frows[0:1, aoff:aoff + NS].bitcast(F32))
                        fresh = smallp.tile([njo, P], F32,
                                            name=f"fs{l}_{half}",
                                            tag="fresh")
                        nc.sync.dma_start(fresh[:], fdt.ap())
                        nc.tensor.transpose(psc[:], fresh[:],
                                            ident[0:njo, 0:njo])
                    else:
                        # peer half: latency-critical (post-collective) -
                        # direct tiny transposes, no DMA round trip
                        for jj in range(njo):
                            nc.tensor.transpose(
                                psc[:, jj:jj + 1],
                                frows[0:1,
                                      aoff + jj * P:aoff + (jj + 1) * P]
                                .bitcast(F32),
                                ident[0:1, 0:1])
                    nc.scalar.activation(ucols[:, csl], psc[:], AF.Exp,
                                         bias=cbias[:])
                    nc.scalar.activation(vcols[:, csl], psc[:], AF.Exp,
                                         scale=0.01)

                    # attention inner loop over this half's j tiles
                    for jj in range(njo):
                        j = half * njo + jj
                        pm = pmp.tile([P, NS], BF16, name=f"pm{l}_{j}",
                                      tag="pm")
                        nc.vector.tensor_scalar(pm[:], Wb[:],
                                                vcols[:, j:j + 1],
                                                ucols[:, j:j + 1],
                                                OP.mult, OP.max)
                        nc.vector.tensor_tensor(pm[:], pm[:], maskT[j][:],
                                                OP.mult)
                        first = j == 0
                        fin = j == nj - 1
                        for ch in range(nch):
                            sl = slice(ch * 512, (ch + 1) * 512)
                            nc.tensor.matmul(psSb[:, sl], onesm[:],
                                             pm[:, sl],
                                             start=first, stop=fin)
                        for ch in range(nch):
                            sl = slice(ch * 512, (ch + 1) * 512)
                            nc.tensor.matmul(psAT[:, sl],
                                             hnat[:, j * P:(j + 1) * P],
                                             pm[:, sl],
                                             start=first, stop=fin)

                # ---- normalize + residual (stay transposed) ----
                lnS = bigp.tile([P, NS], F32, name=f"lnS{l}", tag="lnS")
                Rb = bigp.tile([P, NS], F32, name=f"Rb{l}", tag="Rb")
                # delta = agg/rowsum in bf16: applied identically on both
                # pair members (exchange ships the same bf16 values), so
                # the pair's view of every row stays bit-consistent.
                delta = bigp.tile([P, NS], BF16, name=f"dl{l}", tag="tmp")
                xTs_new = xp.tile([P, NS], F32R, name=f"xTs{l + 1}", tag="xTs")
                for ch in range(nch):
                    sl = slice(ch * 512, (ch + 1) * 512)
                    nc.scalar.activation(lnS[:, sl], psSb[:, sl], AF.Ln)
                    nc.scalar.activation(Rb[:, sl], lnS[:, sl], AF.Exp,
                                         scale=-1.0)
                    nc.vector.tensor_tensor(delta[:, sl], psAT[:, sl],
                                            Rb[:, sl], OP.mult)
                    nc.vector.tensor_tensor(xTs_new[:, sl], delta[:, sl],
                                            xTs[:, sl], OP.add)
                    if not last:
                        nc.sync.dma_start(ag_in[l].ap()[:, sl],
                                          delta[:, sl])
                x_own_old = xTs
                xTs = xTs_new

                if not last:
                    nc.gpsimd.collective_compute(
                        "AllReduce", OP.add, replica_groups=pair_groups,
                        ins=[ag_in[l].ap()], outs=[ag_out[l].ap()])
                    dsum = xp.tile([P, NS], BF16, name=f"dsum{l}", tag="xsum")
                    x1p = bigp.tile([P, NS], F32, name=f"x1p{l}", tag="lnS")
                    x1_old = x1
                    x1 = xp.tile([P, NS], F32R, name=f"x1_{l + 1}", tag="x1")
                    for ch in range(nch):
                        sl = slice(ch * 512, (ch + 1) * 512)
                        # x1_old - own delta: runs during the collective
                        nc.vector.tensor_tensor(x1p[:, sl], x1_old[:, sl],
                                                delta[:, sl], OP.subtract)
                    for ch in range(nch):
                        sl = slice(ch * 512, (ch + 1) * 512)
                        nc.gpsimd.dma_start(dsum[:, sl],
                                            ag_out[l].ap()[:, sl])
                        nc.vector.tensor_tensor(x1[:, sl], x1p[:, sl],
                                                dsum[:, sl], OP.add)

            # ---- final linear: outT = relu(Wt.x + bt) ----
            for g in range(nH):
                ob = outp.tile([P, NS], F32, name=f"ob{g}", tag="ob")
                for ch in range(nch):
                    sl = slice(ch * 512, (ch + 1) * 512)
                    ps = rot.tile([P, 512], F32, name=f"o{g}_{ch}", tag="rot")
                    nc.tensor.matmul(ps[:],
                                     WtTt[:, g * P:(g + 1) * P],
                                     xTs[:, sl],
                                     start=True, stop=True)
                    nc.vector.tensor_scalar(ob[:, sl], ps[:],
                                            btpt[:, g:g + 1], 0.0,
                                            OP.add, OP.max)
                    nc.sync.dma_start(
                        out_ext.ap()[g * P:(g + 1) * P, sl], ob[:, sl])

    if legalize:
        _legalize_waits(nc)
    return nc


def make_in_maps(x, adj, Ws, bs, avs, Wt, bt, num_cores, NS):
    """Per-core input dicts. Core c -> (graph c//2, row-half c%2).
    Local-j convention: each core's atom axis is permuted so its own
    half comes first; mask columns follow the same permutation."""
    B, N, D = x.shape
    H = Wt.shape[0]
    nH = H // P
    x = np.ascontiguousarray(x, np.float32)
    shared = {"WtT": np.ascontiguousarray(np.asarray(Wt, np.float32).T),
              "btp": np.ascontiguousarray(
                  np.asarray(bt, np.float32).reshape(nH, P).T)}
    for l, (W, b, a) in enumerate(zip(Ws, bs, avs)):
        shared[f"WT{l}"] = np.ascontiguousarray(np.asarray(W, np.float32).T)
        shared[f"bv{l}"] = np.ascontiguousarray(
            np.asarray(b, np.float32).reshape(D, 1))
        avm = np.zeros((D, 33), np.float32)
        avm[:, 0] = np.asarray(a, np.float32)[D:, 0]
        avm[:, 32] = np.asarray(a, np.float32)[:D, 0]
        shared[f"av{l}"] = avm
    in_maps = []
    for c in range(num_cores):
        b, s = c // 2, c % 2
        own = slice(s * NS, (s + 1) * NS)
        peer = slice((1 - s) * NS, (2 - s) * NS)
        m = dict(shared)
        m["xTs"] = np.ascontiguousarray(x[b, own].T)
        m["xTo"] = np.ascontiguousarray(x[b, peer].T)
        # mask[j_local, i_own] = adj[b, own_i, j_global] with own atoms first
        adjb = (np.asarray(adj[b]) > 0)
        mk = np.concatenate([adjb[own, own].T, adjb[own, peer].T], axis=0)
        m["maskT"] = np.ascontiguousarray(mk.astype(ml_dtypes.bfloat16))
        in_maps.append(m)
    return in_maps


_NC_CACHE = {}


def kernel(x, adj, W0, b0, W1, b1, W2, b2, a0, a1, a2, Wt, bt):
    B, N, D = 4, 2048, 128
    H = 256
    NUM_CORES = 8
    NS = N // 2
    pair_groups = [[2 * i, 2 * i + 1] for i in range(NUM_CORES // 2)]

    key = (N, NS, D, H, NUM_CORES)
    if key not in _NC_CACHE:
        _NC_CACHE[key] = build_gat_nc(N, NS, D, H, NUM_CORES, pair_groups)
    nc = _NC_CACHE[key]

    in_maps = make_in_maps(np.asarray(x), np.asarray(adj),
                           [W0, W1, W2], [b0, b1, b2], [a0, a1, a2],
                           np.asarray(Wt), np.asarray(bt), NUM_CORES, NS)
    res = run_bass_kernel_spmd(nc, in_maps, list(range(NUM_CORES))).results
    out = np.empty((B, N, H), np.float32)
    for c in range(NUM_CORES):
        b, s = c // 2, c % 2
        out[b, s * NS:(s + 1) * NS, :] = res[c]["out_s"].T
    return out


# revision 32
# speedup vs baseline: 1.1753x; 1.1753x over previous
"""GAT (3-layer graph attention + final linear) Trainium2 Bass kernel, v2.

Problem: B=4 graphs, N=2048 atoms, D=128, H=256.
  per layer: h = relu(x @ W.T + b); e_ij = leaky_relu(f1_i + f2_j, 0.01)
  masked by adj; att = softmax_j(e); x = x + att @ h.
  final: relu(x @ Wt.T + bt).

Sharding: 8 cores; core c -> (graph b=c//2, row-half s=c%2). Each core
computes attention for its own 1024 rows (i), over all 2048 atoms (j).

Key design (vs v1 baseline at 607us):
  - Rank-1 softmax restructure: divide row i of the numerator by
    exp(f1_i) (cancels in softmax).  With z = f1_i + f2_j,
      exp(leaky(z)) / e^{f1_i + C} = max(u_j, v_j * w_i)
    where u = exp(f2 - C), v = exp(.01 f2), w = exp(-.99 f1 - C),
    C = local max f2 (any row-constant cancels; C only bounds ranges).
    So the NxN attention numerator needs NO exp, NO logit matmul and NO
    mask-preload matmul: one fused DVE tensor_scalar (mult by v-col,
    max with u-col over a broadcast w tile) plus one DVE mask multiply.
    PE streams each masked-prob tile twice (aggregation + row-sum).
  - All dense matmuls in float32r (1 cy/row vs fp32's 2x4) or bf16.
  - Mask shipped from HOST as ready-to-use transposed bf16 0/1 tiles
    (adj[b][rows,:].T) - no device-side int32 convert/transpose pass,
    and half the HBM traffic of int32.
  - Local-j index remap (host permutes mask columns so each core's own
    atoms come first) makes the program fully SPMD-uniform, and the
    inter-layer exchange is a pair AllReduce(add) of the updated own
    rows; peer half = pairsum - own (one DVE subtract). Own-half
    attention work overlaps the collective on every core.
  - A tiny dummy AllReduce at kernel start absorbs the ~80us
    first-collective warm-up under layer-0 compute.
  - x state kept transposed end-to-end; normalization applied in
    transposed space via a gpsimd partition_broadcast of 1/rowsum.
"""

import numpy as np
import ml_dtypes

import concourse.bass as bass
import concourse.mybir as mybir
import concourse.tile as tile
from concourse import library_config, masks
from concourse.bass_utils import run_bass_kernel_spmd

P = 128
F32 = mybir.dt.float32
F32R = mybir.dt.float32r
BF16 = mybir.dt.bfloat16
AF = mybir.ActivationFunctionType
OP = mybir.AluOpType

# Static exponent shift: bounds exp() ranges (cancels exactly in softmax).
# f2 stays well under this for the GAT data distribution (observed max ~25).
CBIAS = -24.0


def _legalize_waits(nc, dma_limit=1, engine_limit=1):
    """Walrus can encode only 1 sem wait on a DMA instruction and ~2 on an
    engine instruction. Move excess waits onto standalone EventSemaphore
    instructions (1 wait each) inserted just before the offender on the
    same engine."""
    counter = [0]

    def split(ins):
        si = ins.sync_info
        if si is None:
            return None
        limit = dma_limit if type(ins).__name__.startswith("InstDMA") \
            else engine_limit
        waits = list(si.on_wait)
        if len(waits) <= limit:
            return None
        keep = waits[-limit:] if limit > 0 else []
        extra = waits[:-limit] if limit > 0 else waits
        evs = []
        for w in extra:
            counter[0] += 1
            evs.append(mybir.InstEventSemaphore(
                name=f"evsplit{counter[0]}", engine=ins.engine,
                sync_info=mybir.SyncInfo(on_wait=[w], on_update=[])))
        ins.sync_info = mybir.SyncInfo(on_wait=keep,
                                       on_update=list(si.on_update))
        return evs

    for f in nc.m.functions:
        for blk in f.blocks:
            new_list = []
            changed = False
            for ins in blk.instructions:
                evs = split(ins)
                if evs:
                    new_list.extend(evs)
                    changed = True
                new_list.append(ins)
            if changed:
                blk.instructions = new_list


def r32(ap):
    return ap.bitcast(F32R)


def build_gat_nc(N, NS, D, H, num_cores, pair_groups, nlayers=3,
                 legalize=True):
    assert D == P and NS % 512 == 0 and N % 512 == 0
    nj = N // P          # j tiles over all atoms (local order: own first)
    njo = nj // 2        # own j tiles
    nch = NS // 512      # 512-chunks over own rows / one atom half
    nH = H // P

    nc = bass.Bass("TRN2", target_bir_lowering=False, debug=False,
                   num_devices=num_cores)

    # ---- I/O ----
    xTs_in = nc.dram_tensor("xTs", [P, NS], F32, kind="ExternalInput")
    xTo_in = nc.dram_tensor("xTo", [P, NS], F32, kind="ExternalInput")
    maskT_in = nc.dram_tensor("maskT", [N, NS], BF16, kind="ExternalInput")
    WT_in = [nc.dram_tensor(f"WT{l}", [D, D], F32, kind="ExternalInput")
             for l in range(nlayers)]
    bv_in = [nc.dram_tensor(f"bv{l}", [D, 1], F32, kind="ExternalInput")
             for l in range(nlayers)]
    av_in = [nc.dram_tensor(f"av{l}", [D, 33], F32, kind="ExternalInput")
             for l in range(nlayers)]
    WtT_in = nc.dram_tensor("WtT", [D, H], F32, kind="ExternalInput")
    btp_in = nc.dram_tensor("btp", [P, nH], F32, kind="ExternalInput")
    out_ext = nc.dram_tensor("out_s", [H, NS], F32, kind="ExternalOutput")

    # DRAM bounce buffers for the pair AllReduce of updated own rows
    ag_in = [nc.dram_tensor(f"ag_in{l}", [P, NS], BF16)
             for l in range(nlayers - 1)]
    ag_out = [nc.dram_tensor(f"ag_out{l}", [P, NS], BF16)
              for l in range(nlayers - 1)]
    dw_in = nc.dram_tensor("dw_in", [1, P], BF16)
    dw_out = nc.dram_tensor("dw_out", [1, P], BF16)
    fd = [nc.dram_tensor(f"fd{i}", [N // P // 2, P], F32)
          for i in range(2 * nlayers)]

    with tile.TileContext(nc) as tc:
        import contextlib
        ctx = contextlib.ExitStack()
        with ctx:
            persist = ctx.enter_context(tc.tile_pool(name="persist", bufs=1))
            hTp = ctx.enter_context(tc.tile_pool(name="hTp", bufs=2))
            hnp = ctx.enter_context(tc.tile_pool(name="hnp", bufs=2))
            xp = ctx.enter_context(tc.tile_pool(name="xp", bufs=2))
            rowp = ctx.enter_context(tc.tile_pool(name="rowp", bufs=2))
            colp = ctx.enter_context(tc.tile_pool(name="colp", bufs=2))
            bigp = ctx.enter_context(tc.tile_pool(name="bigp", bufs=2))
            pmp = ctx.enter_context(tc.tile_pool(name="pmp", bufs=4))
            outp = ctx.enter_context(tc.tile_pool(name="outp", bufs=2))
            smallp = ctx.enter_context(tc.tile_pool(name="smallp", bufs=4))
            rot = ctx.enter_context(
                tc.tile_pool(name="rot", bufs=4, space="PSUM"))
            attp = ctx.enter_context(
                tc.tile_pool(name="attp", bufs=1, space="PSUM"))
            spp = ctx.enter_context(
                tc.tile_pool(name="spp", bufs=1, space="PSUM"))

            ident = persist.tile([P, P], F32)
            masks.make_identity(nc, ident[:])
            identR = persist.tile([P, P], F32R)
            nc.vector.tensor_copy(identR[:], ident[:])
            onesf = persist.tile([33, P], F32)
            nc.vector.memset(onesf[:], 1.0)
            onesr = persist.tile([33, P], F32R)
            nc.vector.tensor_copy(onesr[:], onesf[:])
            onesm = persist.tile([P, P], BF16)
            nc.vector.memset(onesm[:], 1.0)
            identb = persist.tile([P, P], BF16)
            masks.make_identity(nc, identb[:])
            cbias = persist.tile([P, 1], F32)
            nc.vector.memset(cbias[:], CBIAS)

            # ---- dummy collective to warm the CC stream ----
            nc.gpsimd.collective_compute(
                "AllReduce", OP.add, replica_groups=pair_groups,
                ins=[dw_in.ap()], outs=[dw_out.ap()])

            # ---- persistent state ----
            maskT = [persist.tile([P, NS], BF16, name=f"maskT{j}",
                                  tag=f"maskT{j}") for j in range(nj)]

            # ---- initial x state (own + peer halves, transposed) ----
            xTs_d = persist.tile([P, NS], F32, name="xTs_d", tag="xTs_d")
            nc.sync.dma_start(xTs_d[:], xTs_in.ap())
            xTo_d = persist.tile([P, NS], F32, name="xTo_d", tag="xTo_d")
            nc.sync.dma_start(xTo_d[:], xTo_in.ap())
            xTs = xp.tile([P, NS], F32R, name="xTs0", tag="xTs")
            nc.vector.tensor_copy(xTs[:], xTs_d[:])
            x1 = xp.tile([P, NS], F32R, name="xTo0", tag="x1")
            nc.vector.tensor_copy(x1[:], xTo_d[:])

            # weights: DMA + DVE launder so matmuls see single-writer tiles
            WT_d = [persist.tile([D, D], F32, name=f"WTd{l}", tag=f"WTd{l}")
                    for l in range(nlayers)]
            bv_d = [persist.tile([D, 1], F32, name=f"bvd{l}", tag=f"bvd{l}")
                    for l in range(nlayers)]
            av_d = [persist.tile([D, 33], F32, name=f"avd{l}", tag=f"avd{l}")
                    for l in range(nlayers)]
            WtT_d = persist.tile([D, H], F32)
            btp_d = persist.tile([P, nH], F32)
            WT = [persist.tile([D, D], F32R, name=f"WTl{l}", tag=f"WTl{l}")
                  for l in range(nlayers)]
            bv = [persist.tile([D, 1], F32, name=f"bvl{l}", tag=f"bvl{l}")
                  for l in range(nlayers)]
            av = [persist.tile([D, 33], F32R, name=f"avl{l}", tag=f"avl{l}")
                  for l in range(nlayers)]
            WtTt = persist.tile([D, H], F32R)
            btpt = persist.tile([P, nH], F32)
            for l in range(nlayers):
                nc.sync.dma_start(WT_d[l][:], WT_in[l].ap())
                nc.sync.dma_start(bv_d[l][:], bv_in[l].ap())
                nc.sync.dma_start(av_d[l][:], av_in[l].ap())
                nc.vector.tensor_copy(WT[l][:], WT_d[l][:])
                nc.vector.tensor_copy(bv[l][:], bv_d[l][:])
                nc.vector.tensor_copy(av[l][:], av_d[l][:])
            nc.sync.dma_start(WtT_d[:], WtT_in.ap())
            nc.sync.dma_start(btp_d[:], btp_in.ap())
            nc.vector.tensor_copy(WtTt[:], WtT_d[:])
            nc.vector.tensor_copy(btpt[:], btp_d[:])
            # masks after x/weights; own half first; spread across queues
            for j in range(nj):
                q = nc.sync if j < njo else nc.scalar
                q.dma_start(maskT[j][:], maskT_in.ap()[j * P:(j + 1) * P, :])

            for l in range(nlayers):
                last = l == nlayers - 1
                hT = hTp.tile([P, N], F32R, name=f"hT{l}", tag="hT")
                hTb = hTp.tile([P, N], BF16, name=f"hTb{l}", tag="hTb")
                frows = rowp.tile([33, N], F32R, name=f"frows{l}", tag="frows")
                ucols = colp.tile([P, nj], F32, name=f"uc{l}", tag="uc")
                vcols = colp.tile([P, nj], F32, name=f"vc{l}", tag="vc")
                hnat = hnp.tile([P, N], BF16, name=f"hnat{l}", tag="hnat")
                psAT = attp.tile([P, NS], F32, name=f"psAT{l}", tag="att")
                psSb = spp.tile([P, NS], F32, name=f"psS{l}", tag="s")
                Wb = bigp.tile([P, NS], BF16, name=f"Wb{l}", tag="Wb")

                # process halves: 0 = own (local data), 1 = peer
                for half in range(2):
                    xsrc = xTs if half == 0 else x1
                    aoff = half * NS  # atom offset in local-j space
                    # chunk-pipelined: h -> f -> (Wb) -> hnat per 512 chunk
                    for ch in range(nch):
                        sl = slice(aoff + ch * 512, aoff + (ch + 1) * 512)
                        ps = rot.tile([P, 512], F32, name=f"h{l}_{half}{ch}",
                                      tag="rot")
                        nc.tensor.matmul(ps[:], WT[l][:],
                                         xsrc[:, ch * 512:(ch + 1) * 512],
                                         start=True, stop=True)
                        nc.scalar.activation(hT[:, sl], ps[:], AF.Relu,
                                             bias=bv[l][:])
                        nc.scalar.activation(hTb[:, sl], ps[:], AF.Relu,
                                             bias=bv[l][:])
                        psf = rot.tile([33, 512], F32,
                                       name=f"f{l}_{half}{ch}", tag="rot")
                        nc.tensor.matmul(psf[:], av[l][:], hT[:, sl],
                                         start=True, stop=True)
                        nc.scalar.copy(frows[:, sl], psf[:])
                        if half == 0:
                            # Wb[p, i] = exp(-0.99 f1_i - C): fp32r K=1
                            # broadcast of raw f1, exp fused in ACT copy.
                            slo = slice(ch * 512, (ch + 1) * 512)
                            psw = rot.tile([P, 512], F32,
                                           name=f"psw{l}_{ch}", tag="rot")
                            nc.tensor.matmul(psw[:], onesr[32:33, :],
                                             frows[32:33, slo],
                                             start=True, stop=True)
                            nc.scalar.activation(Wb[:, slo], psw[:], AF.Exp,
                                                 bias=cbias[:], scale=-0.99)
                        pst = rot.tile([P, 512], BF16,
                                       name=f"ht{l}_{half}{ch}", tag="rot")
                        for q in range(4):
                            j = half * njo + ch * 4 + q
                            nc.tensor.transpose(pst[:, q * P:(q + 1) * P],
                                                hTb[:, j * P:(j + 1) * P],
                                                identb[:])
                        nc.scalar.copy(
                            hnat[:, aoff + ch * 512:aoff + (ch + 1) * 512],
                            pst[:])

                    csl = slice(half * njo, (half + 1) * njo)
                    psc = rot.tile([P, njo], F32, name=f"psc{l}_{half}",
                                   tag="rot")
                    if half == 0 and l > 0:
                        # own half: f2 row -> cols via DRAM-bounce reshape +
                        # one 8-row transpose (latency-tolerant: overlaps the
                        # previous layer's peer inner loop). Layer 0 has no
                        # such overlap window - use direct transposes there.
                        fdt = fd[2 * l + half]
                        nc.sync.dma_start(
                            fdt.ap().rearrange("a b -> (a b)").unsqueeze(0),
                            frows[0:1, aoff:aoff + NS].bitcast(F32))
                        fresh = smallp.tile([njo, P], F32,
                                            name=f"fs{l}_{half}",
                                            tag="fresh")
                        nc.sync.dma_start(fresh[:], fdt.ap())
                        nc.tensor.transpose(psc[:], fresh[:],
                                            ident[0:njo, 0:njo])
                    else:
                        # peer half: latency-critical (post-collective) -
                        # direct tiny transposes, no DMA round trip
                        for jj in range(njo):
                            nc.tensor.transpose(
                                psc[:, jj:jj + 1],
                                frows[0:1,
                                      aoff + jj * P:aoff + (jj + 1) * P]
                                .bitcast(F32),
                                ident[0:1, 0:1])
                    nc.scalar.activation(ucols[:, csl], psc[:], AF.Exp,
                                         bias=cbias[:])
                    nc.scalar.activation(vcols[:, csl], psc[:], AF.Exp,
                                         scale=0.01)

                    # attention inner loop over this half's j tiles
                    for jj in range(njo):
                        j = half * njo + jj
                        pm = pmp.tile([P, NS], BF16, name=f"pm{l}_{j}",
                                      tag="pm")
                        nc.vector.tensor_scalar(pm[:], Wb[:],
                                                vcols[:, j:j + 1],
                                                ucols[:, j:j + 1],
                                                OP.mult, OP.max)
                        nc.vector.tensor_tensor(pm[:], pm[:], maskT[j][:],
                                                OP.mult)
                        first = j == 0
                        fin = j == nj - 1
                        for ch in range(nch):
                            sl = slice(ch * 512, (ch + 1) * 512)
                            nc.tensor.matmul(psSb[:, sl], onesm[:],
                                             pm[:, sl],
                                             start=first, stop=fin)
                        for ch in range(nch):
                            sl = slice(ch * 512, (ch + 1) * 512)
                            nc.tensor.matmul(psAT[:, sl],
                                             hnat[:, j * P:(j + 1) * P],
                                             pm[:, sl],
                                             start=first, stop=fin)

                # ---- normalize + residual (stay transposed) ----
                lnS = bigp.tile([P, NS], F32, name=f"lnS{l}", tag="lnS")
                Rb = bigp.tile([P, NS], F32, name=f"Rb{l}", tag="Rb")
                # delta = agg/rowsum in bf16: applied identically on both
                # pair members (exchange ships the same bf16 values), so
                # the pair's view of every row stays bit-consistent.
                delta = bigp.tile([P, NS], BF16, name=f"dl{l}", tag="tmp")
                xTs_new = xp.tile([P, NS], F32R, name=f"xTs{l + 1}", tag="xTs")
                for ch in range(nch):
                    sl = slice(ch * 512, (ch + 1) * 512)
                    nc.scalar.activation(lnS[:, sl], psSb[:, sl], AF.Ln)
                    nc.scalar.activation(Rb[:, sl], lnS[:, sl], AF.Exp,
                                         scale=-1.0)
                    nc.vector.tensor_tensor(delta[:, sl], psAT[:, sl],
                                            Rb[:, sl], OP.mult)
                    nc.vector.tensor_tensor(xTs_new[:, sl], delta[:, sl],
                                            xTs[:, sl], OP.add)
                    if not last:
                        nc.sync.dma_start(ag_in[l].ap()[:, sl],
                                          delta[:, sl])
                x_own_old = xTs
                xTs = xTs_new

                if not last:
                    nc.gpsimd.collective_compute(
                        "AllReduce", OP.add, replica_groups=pair_groups,
                        ins=[ag_in[l].ap()], outs=[ag_out[l].ap()])
                    dsum = xp.tile([P, NS], BF16, name=f"dsum{l}", tag="xsum")
                    x1p = bigp.tile([P, NS], F32, name=f"x1p{l}", tag="lnS")
                    x1_old = x1
                    x1 = xp.tile([P, NS], F32R, name=f"x1_{l + 1}", tag="x1")
                    for ch in range(nch):
                        sl = slice(ch * 512, (ch + 1) * 512)
                        # x1_old - own delta: runs during the collective
                        nc.vector.tensor_tensor(x1p[:, sl], x1_old[:, sl],
                                                delta[:, sl], OP.subtract)
                    for ch in range(nch):
                        sl = slice(ch * 512, (ch + 1) * 512)
                        nc.gpsimd.dma_start(dsum[:, sl],
                                            ag_out[l].ap()[:, sl])
                        nc.vector.tensor_tensor(x1[:, sl], x1p[:, sl],
                                                dsum[:, sl], OP.add)

            # ---- final linear: outT = relu(Wt.x + bt) ----
            for g in range(nH):
                ob = outp.tile([P, NS], F32, name=f"ob{g}", tag="ob")
                for ch in range(nch):
                    sl = slice(ch * 512, (ch + 1) * 512)
                    ps = rot.tile([P, 512], F32, name=f"o{g}_{ch}", tag="rot")
                    nc.tensor.matmul(ps[:],
                                     WtTt[:, g * P:(g + 1) * P],
                                     xTs[:, sl],
                                     start=True, stop=True)
                    nc.vector.tensor_scalar(ob[:, sl], ps[:],
                                            btpt[:, g:g + 1], 0.0,
                                            OP.add, OP.max)
                    nc.sync.dma_start(
                        out_ext.ap()[g * P:(g + 1) * P, sl], ob[:, sl])

    if legalize:
        _legalize_waits(nc)
    return nc


def make_in_maps(x, adj, Ws, bs, avs, Wt, bt, num_cores, NS):
    """Per-core input dicts. Core c -> (graph c//2, row-half c%2).
    Local-j convention: each core's atom axis is permuted so its own
    half comes first; mask columns follow the same permutation."""
    B, N, D = x.shape
    H = Wt.shape[0]
    nH = H // P
    x = np.ascontiguousarray(x, np.float32)
    shared = {"WtT": np.ascontiguousarray(np.asarray(Wt, np.float32).T),
              "btp": np.ascontiguousarray(
                  np.asarray(bt, np.float32).reshape(nH, P).T)}
    for l, (W, b, a) in enumerate(zip(Ws, bs, avs)):
        shared[f"WT{l}"] = np.ascontiguousarray(np.asarray(W, np.float32).T)
        shared[f"bv{l}"] = np.ascontiguousarray(
            np.asarray(b, np.float32).reshape(D, 1))
        avm = np.zeros((D, 33), np.float32)
        avm[:, 0] = np.asarray(a, np.float32)[D:, 0]
        avm[:, 32] = np.asarray(a, np.float32)[:D, 0]
        shared[f"av{l}"] = avm
    in_maps = []
    for c in range(num_cores):
        b, s = c // 2, c % 2
        own = slice(s * NS, (s + 1) * NS)
        peer = slice((1 - s) * NS, (2 - s) * NS)
        m = dict(shared)
        m["xTs"] = np.ascontiguousarray(x[b, own].T)
        m["xTo"] = np.ascontiguousarray(x[b, peer].T)
        # mask[j_local, i_own] = adj[b, own_i, j_global] with own atoms first
        adjb = (np.asarray(adj[b]) > 0)
        mk = np.concatenate([adjb[own, own].T, adjb[own, peer].T], axis=0)
        m["maskT"] = np.ascontiguousarray(mk.astype(ml_dtypes.bfloat16))
        in_maps.append(m)
    return in_maps


_NC_CACHE = {}


def kernel(x, adj, W0, b0, W1, b1, W2, b2, a0, a1, a2, Wt, bt):
    B, N, D = 4, 2048, 128
    H = 256
    NUM_CORES = 8
    NS = N // 2
    pair_groups = [[2 * i, 2 * i + 1] for i in range(NUM_CORES // 2)]

    key = (N, NS, D, H, NUM_CORES)
    if key not in _NC_CACHE:
        _NC_CACHE[key] = build_gat_nc(N, NS, D, H, NUM_CORES, pair_groups)
    nc = _NC_CACHE[key]

    in_maps = make_in_maps(np.asarray(x), np.asarray(adj),
                           [W0, W1, W2], [b0, b1, b2], [a0, a1, a2],
                           np.asarray(Wt), np.asarray(bt), NUM_CORES, NS)
    res = run_bass_kernel_spmd(nc, in_maps, list(range(NUM_CORES))).results
    out = np.empty((B, N, H), np.float32)
    for c in range(NUM_CORES):
        b, s = c // 2, c % 2
        out[b, s * NS:(s + 1) * NS, :] = res[c]["out_s"].T
    return out
